# revision 9
# baseline (speedup 1.0000x reference)
"""Chopfield attention (complex QKV + real-part softmax attention) on 8
Trainium2 NeuronCores — collective-free restructure.

Algebra: Z = BETA*Re(conj(R@W_Q) @ (Y@W_K)^T) = Re(conj(R) @ G @ Y^T)
with G = BETA * conj(W_Q) @ W_K^T precomputed on HOST (weights only).
This kills the K projection and its AllGather. The output side uses
out = A @ (Y@W_V) = (A@Y) @ W_V, computed transposed:
out^T = W_V^T @ (Y^T @ A^T), killing the V projection AllGather. Each
core owns 512 query rows (R shard) and reads the full Y from its own
DRAM copy — zero inter-core communication.

Device math per core (q = 512 local rows):
  T^T = (conj(R) @ G)^T          conj-Karatsuba, 3-pass fp16 hi/lo
  Z   = T_re @ Y_re^T + (-T_im) @ Y_im^T     2-pass (T hi/lo, Y hi)
  A   = softmax(Z) streamed over 8 key chunks (online max, late rescale)
  U   = Y^T @ A^T                fp16 single-pass (stationary = Y tiles)
  o^T = W_V^T @ U                Karatsuba, fp16 single-pass

Precision: validated by host simulation (sim.py) at L2 rel err 9.7e-3
vs the 2e-2 gate; the score chain keeps ~fp32 accuracy via fp16 hi/lo
splits with exact PE products and fp32 PSUM accumulation.
"""

import numpy as np

import concourse.bacc as bacc
import concourse.mybir as mybir
import concourse.tile as tile
from concourse.bass_utils import run_bass_kernel_spmd

BETA = 0.03125
P = 128
FP16 = mybir.dt.float16
FP32 = mybir.dt.float32
X = mybir.AxisListType.X

# variant order in rt_all / g_all free axis
V6 = {("re", "h"): 0, ("re", "l"): 1, ("im", "h"): 2, ("im", "l"): 3,
      ("s", "h"): 4, ("s", "l"): 5}


class Cfg:
    def __init__(self, N=4096, M=4096, D=1024, NC=8):
        self.N, self.M, self.D, self.NC = N, M, D, NC
        self.NL = N // NC          # local query rows (512)
        self.DT = D // P           # d tiles (8)
        self.QTS = self.NL // P    # local query partition-tiles (4)
        self.MC = 8                # key chunks
        self.MCW = M // self.MC    # chunk width (512)
        self.MT = M // P           # global key partition-tiles (32)


def build(cfg: Cfg, reps: int = 1, stop_after: str | None = None):
    c = cfg
    nc = bacc.Bacc("TRN2", target_bir_lowering=False, debug=False, num_devices=c.NC)

    def din(name, shape):
        return nc.dram_tensor(name, shape, FP16, kind="ExternalInput")

    rt_all = din("rt_all", [P, 6 * c.DT * c.NL])          # R^T variants
    g_all = din("g_all", [c.DT, P, 6 * c.D])              # G stationary tiles
    ytr = din("ytr", [c.MC, P, c.DT * c.MCW])             # Y_re^T hi, chunked
    yti = din("yti", [c.MC, P, c.DT * c.MCW])             # Y_im^T hi, chunked
    y_all = din("y_all", [2 * c.DT, P, c.MT * P])         # Y re/im stationary tiles
    wv_all = din("wv_all", [c.DT, P, 3 * c.D])            # W_V re/im/s stationary
    ident = din("ident", [P, P])

    o_ret = nc.dram_tensor("o_ret", [c.DT, P, c.NL], FP32, kind="ExternalOutput")
    o_imt = nc.dram_tensor("o_imt", [c.DT, P, c.NL], FP32, kind="ExternalOutput")

    with tile.TileContext(nc) as tc:
        with (
            tc.tile_pool(name="pers", bufs=1) as pers,
            tc.tile_pool(name="ps", bufs=1, space="PSUM") as ps,
        ):
            ident_sb = pers.tile([P, P], FP16, tag="ident")
            nc.sync.dma_start(ident_sb[:], ident.ap())

            def emit(rep):
                # ---------------- phase A: T^T = (conj(R) @ G)^T ----------
                pT = tc.alloc_tile_pool(name=f"pT{rep}", bufs=1)
                pA = tc.alloc_tile_pool(name=f"pA{rep}", bufs=1)
                pW = tc.alloc_tile_pool(name=f"pW{rep}", bufs=1)

                rt_sb = pA.tile([P, 6 * c.DT * c.NL], FP16, tag="rt", name=f"rt_{rep}")
                nc.sync.dma_start(rt_sb[:], rt_all.ap())

                def rts(v, ki):
                    o = V6[v] * c.DT * c.NL + ki * c.NL
                    return rt_sb[:, o : o + c.NL]

                t_sb = {}
                for comp in ("re", "im"):
                    for lvl in ("h", "l"):
                        t_sb[comp, lvl] = pT.tile(
                            [P, c.DT * c.NL], FP16, tag=f"t{comp}{lvl}",
                            name=f"t_{comp}_{lvl}_{rep}")

                for dout in range(c.DT):
                    gsl = pA.tile([P, 6 * c.D], FP16, tag="gsl", bufs=2)
                    nc.scalar.dma_start(gsl[:], g_all.ap()[dout])

                    def gs(v, ki):
                        o = V6[v] * c.D + ki * P
                        return gsl[:, o : o + P]

                    m = []
                    for rv in ("re", "im", "s"):
                        pt = ps.tile([P, 512], FP32, tag="ps", bufs=6)
                        m.append(pt[:, : c.NL])
                        nmm = c.DT * 3
                        i = 0
                        for ki in range(c.DT):
                            for rl, gl in (("h", "h"), ("l", "h"), ("h", "l")):
                                nc.tensor.matmul(
                                    m[-1], gs((rv, gl), ki), rts((rv, rl), ki),
                                    start=(i == 0), stop=(i == nmm - 1))
                                i += 1
                    # Tre = m1 + m2 ; Tim_neg = m1 - m2 - m3   (conj-Karatsuba)
                    m2s = pW.tile([P, c.NL], FP32, tag="m2s", bufs=2)
                    nc.vector.tensor_copy(m2s[:], m[1])
                    tre = pW.tile([P, c.NL], FP32, tag="tre", bufs=2)
                    nc.vector.tensor_add(tre[:], m[0], m2s[:])
                    v1 = pW.tile([P, c.NL], FP32, tag="v1", bufs=2)
                    nc.vector.tensor_sub(v1[:], m[0], m2s[:])
                    timn = pW.tile([P, c.NL], FP32, tag="timn", bufs=2)
                    nc.vector.tensor_sub(timn[:], v1[:], m[2])
                    for comp, src in (("re", tre), ("im", timn)):
                        hi = t_sb[comp, "h"][:, dout * c.NL : (dout + 1) * c.NL]
                        lo = t_sb[comp, "l"][:, dout * c.NL : (dout + 1) * c.NL]
                        nc.vector.tensor_copy(hi, src[:])
                        nc.vector.tensor_sub(lo, src[:], hi)
                pW.release()
                pA.release()
                if stop_after == "tproj":
                    pT.release()
                    return

                # ---------------- phase B: Z chunks + online softmax ------
                pP = tc.alloc_tile_pool(name=f"pP{rep}", bufs=1)
                pY = tc.alloc_tile_pool(name=f"pY{rep}", bufs=1)
                p_sb = [pP.tile([P, c.M], FP16, tag=f"p{qt}", name=f"p_{qt}_{rep}")
                        for qt in range(c.QTS)]
                cm = [pP.tile([P, c.MC], FP32, tag=f"cm{qt}", name=f"cm_{qt}_{rep}")
                      for qt in range(c.QTS)]
                ncm = [pP.tile([P, c.MC], FP32, tag=f"ncm{qt}", name=f"ncm_{qt}_{rep}")
                       for qt in range(c.QTS)]
                csum = [pP.tile([P, c.MC], FP32, tag=f"cs{qt}", name=f"cs_{qt}_{rep}")
                        for qt in range(c.QTS)]

                for mc in range(c.MC):
                    ybr = pY.tile([P, c.DT * c.MCW], FP16, tag="ybr", bufs=3)
                    nc.gpsimd.dma_start(ybr[:], ytr.ap()[mc])
                    ybi = pY.tile([P, c.DT * c.MCW], FP16, tag="ybi", bufs=3)
                    nc.sync.dma_start(ybi[:], yti.ap()[mc])
                    for qt in range(c.QTS):
                        zp = ps.tile([P, 512], FP32, tag="ps", bufs=6)
                        zacc = zp[:, : c.MCW]
                        nmm = 2 * c.DT * 2
                        i = 0
                        for yb, comp in ((ybr, "re"), (ybi, "im")):
                            for ki in range(c.DT):
                                for lvl in ("h", "l"):
                                    nc.tensor.matmul(
                                        zacc,
                                        t_sb[comp, lvl][:, ki * c.NL + qt * P : ki * c.NL + (qt + 1) * P],
                                        yb[:, ki * c.MCW : (ki + 1) * c.MCW],
                                        start=(i == 0), stop=(i == nmm - 1))
                                    i += 1
                        nc.vector.reduce_max(cm[qt][:, mc : mc + 1], zacc, axis=X)
                        nc.vector.tensor_scalar_mul(
                            ncm[qt][:, mc : mc + 1], cm[qt][:, mc : mc + 1], -1.0)
                        psl = p_sb[qt][:, mc * c.MCW : (mc + 1) * c.MCW]
                        nc.scalar.activation(
                            psl, zacc, mybir.ActivationFunctionType.Exp,
                            bias=ncm[qt][:, mc : mc + 1], scale=1.0)
                        nc.vector.reduce_sum(csum[qt][:, mc : mc + 1], psl, axis=X)
                pY.release()
                if stop_after == "scores":
                    pP.release()
                    pT.release()
                    return

                # ---------------- phase C: rescale + transpose A ----------
                pPT = tc.alloc_tile_pool(name=f"pPT{rep}", bufs=1)
                for qt in range(c.QTS):
                    ngm = pP.tile([P, 1], FP32, tag=f"ngm{qt}")
                    nc.vector.tensor_reduce(
                        ngm[:], ncm[qt][:], op=mybir.AluOpType.min, axis=X)
                    fac = pP.tile([P, c.MC], FP32, tag=f"fac{qt}")
                    nc.scalar.activation(
                        fac[:], ncm[qt][:], mybir.ActivationFunctionType.Exp,
                        bias=ngm[:, 0:1], scale=-1.0)
                    fs = pP.tile([P, c.MC], FP32, tag=f"fs{qt}")
                    nc.vector.tensor_mul(fs[:], fac[:], csum[qt][:])
                    ssum = pP.tile([P, 1], FP32, tag=f"ss{qt}")
                    nc.vector.reduce_sum(ssum[:], fs[:], axis=X)
                    rc = pP.tile([P, 1], FP32, tag=f"rc{qt}")
                    nc.vector.reciprocal(rc[:], ssum[:])
                    fac2 = pP.tile([P, c.MC], FP32, tag=f"f2{qt}")
                    nc.vector.tensor_scalar_mul(fac2[:], fac[:], rc[:, 0:1])
                    for mc in range(c.MC):
                        psl = p_sb[qt][:, mc * c.MCW : (mc + 1) * c.MCW]
                        nc.vector.tensor_scalar_mul(psl, psl, fac2[:, mc : mc + 1])

                pt_sb = [pPT.tile([P, c.NL], FP16, tag=f"pt{mt}", name=f"pt_{mt}_{rep}")
                         for mt in range(c.MT)]
                for mt in range(c.MT):
                    tp = ps.tile([P, 512], FP16, tag="dsc", bufs=2)
                    tacc = tp[:, : c.NL]
                    for qt in range(c.QTS):
                        nc.tensor.matmul(
                            tacc[:, qt * P : (qt + 1) * P],
                            p_sb[qt][:, mt * P : (mt + 1) * P],
                            ident_sb[:], start=True, stop=True, is_transpose=True)
                    nc.vector.tensor_copy(pt_sb[mt][:], tacc)
                if stop_after == "transp":
                    pPT.release()
                    pP.release()
                    pT.release()
                    return

                # ---------------- phase D: U = Y^T @ A^T ------------------
                pU = tc.alloc_tile_pool(name=f"pU{rep}", bufs=1)
                pYB = tc.alloc_tile_pool(name=f"pYB{rep}", bufs=1)
                u16 = {comp: pU.tile([P, c.DT * c.NL], FP16, tag=f"u{comp}",
                                     name=f"u_{comp}_{rep}")
                       for comp in ("re", "im")}
                us16 = pU.tile([P, c.DT * c.NL], FP16, tag="us", name=f"us_{rep}")
                for ci, comp in enumerate(("re", "im")):
                    for dt in range(c.DT):
                        yb = pYB.tile([P, c.MT * P], FP16, tag=f"yb{ci}", bufs=2)
                        eng = nc.gpsimd if ci == 0 else nc.sync
                        eng.dma_start(yb[:], y_all.ap()[ci * c.DT + dt])
                        up = ps.tile([P, 512], FP32, tag="ps", bufs=6)
                        ua = up[:, : c.NL]
                        for mt in range(c.MT):
                            nc.tensor.matmul(
                                ua, yb[:, mt * P : (mt + 1) * P], pt_sb[mt][:],
                                start=(mt == 0), stop=(mt == c.MT - 1))
                        nc.vector.tensor_copy(
                            u16[comp][:, dt * c.NL : (dt + 1) * c.NL], ua)
                for dt in range(c.DT):
                    sl = slice(dt * c.NL, (dt + 1) * c.NL)
                    nc.vector.tensor_add(us16[:, sl], u16["re"][:, sl], u16["im"][:, sl])
                pYB.release()
                if stop_after == "u":
                    pU.release()
                    pPT.release()
                    pP.release()
                    pT.release()
                    return

                # ---------------- phase E: o^T = W_V^T @ U ----------------
                pE = tc.alloc_tile_pool(name=f"pE{rep}", bufs=1)
                for dout in range(c.DT):
                    wsl = pE.tile([P, 3 * c.D], FP16, tag="wsl", bufs=2)
                    nc.scalar.dma_start(wsl[:], wv_all.ap()[dout])
                    m = []
                    for wi, usrc in ((0, u16["re"]), (1, u16["im"]), (2, us16)):
                        pt = ps.tile([P, 512], FP32, tag="ps", bufs=6)
                        m.append(pt[:, : c.NL])
                        for ki in range(c.DT):
                            nc.tensor.matmul(
                                m[-1],
                                wsl[:, wi * c.D + ki * P : wi * c.D + (ki + 1) * P],
                                usrc[:, ki * c.NL : (ki + 1) * c.NL],
                                start=(ki == 0), stop=(ki == c.DT - 1))
                    m2s = pE.tile([P, c.NL], FP32, tag="em2s", bufs=2)
                    nc.vector.tensor_copy(m2s[:], m[1])
                    ore = pE.tile([P, c.NL], FP32, tag="ore", bufs=2)
                    nc.vector.tensor_sub(ore[:], m[0], m2s[:])
                    nc.sync.dma_start(o_ret.ap()[dout], ore[:])
                    t1 = pE.tile([P, c.NL], FP32, tag="t1", bufs=2)
                    nc.vector.tensor_sub(t1[:], m[2], m2s[:])
                    oim = pE.tile([P, c.NL], FP32, tag="oim", bufs=2)
                    nc.vector.tensor_sub(oim[:], t1[:], m[0])
                    nc.gpsimd.dma_start(o_imt.ap()[dout], oim[:])
                pE.release()
                pU.release()
                pPT.release()
                pP.release()
                pT.release()

            for rep in range(reps):
                emit(rep)

    nc.compile()
    return nc


def _split16(x):
    h = x.astype(np.float16)
    l = (x - h.astype(np.float32)).astype(np.float16)
    return h, l


def _stat_swizzle(w16, DT):
    # [din, dout] -> [dout_tile, p(din%128), din_tile*128 + c]
    return np.ascontiguousarray(
        w16.reshape(DT, P, DT, P).transpose(2, 1, 0, 3).reshape(DT, P, DT * P))


def prep_inputs(cfg, R_re, R_im, Y_re, Y_im, W_Q_re, W_Q_im, W_K_re, W_K_im,
                W_V_re, W_V_im):
    c = cfg
    f32 = np.float32
    DT, MC, MCW, MT = c.DT, c.MC, c.MCW, c.MT

    WQ = np.asarray(W_Q_re, f32) - 1j * np.asarray(W_Q_im, f32)
    WK = np.asarray(W_K_re, f32) + 1j * np.asarray(W_K_im, f32)
    G = BETA * (WQ.astype(np.complex64) @ WK.astype(np.complex64).T)
    Gre = np.ascontiguousarray(G.real, f32)
    Gim = np.ascontiguousarray(G.imag, f32)
    Gs = Gre + Gim

    g_parts = []
    for arr in (Gre, Gim, Gs):
        h, l = _split16(arr)
        g_parts += [_stat_swizzle(h, DT), _stat_swizzle(l, DT)]
    # order must match V6: re_h, re_l, im_h, im_l, s_h, s_l
    g_all = np.ascontiguousarray(np.concatenate(g_parts, axis=2))

    wv_re = np.asarray(W_V_re, f32)
    wv_im = np.asarray(W_V_im, f32)
    wv_all = np.ascontiguousarray(np.concatenate(
        [_stat_swizzle(a.astype(np.float16), DT)
         for a in (wv_re, wv_im, wv_re + wv_im)], axis=2))

    def yt_chunk(ymat):  # [M, D] f32 -> Y^T hi chunked [MC, P, DT*MCW]
        yt = np.ascontiguousarray(np.asarray(ymat, f32).T).astype(np.float16)
        return np.ascontiguousarray(
            yt.reshape(DT, P, MC, MCW).transpose(2, 1, 0, 3).reshape(MC, P, DT * MCW))

    ytr_h = yt_chunk(Y_re)
    yti_h = yt_chunk(Y_im)

    def y_stat(ymat):  # [M, D] -> [DT, P, MT*P] stationary tiles
        y16 = np.asarray(ymat, f32).astype(np.float16)
        return np.ascontiguousarray(
            y16.reshape(MT, P, DT, P).transpose(2, 1, 0, 3).reshape(DT, P, MT * P))

    y_all = np.ascontiguousarray(
        np.concatenate([y_stat(Y_re), y_stat(Y_im)], axis=0))

    ident = np.eye(P, dtype=np.float16)

    shared = {"g_all": g_all, "wv_all": wv_all, "ytr": ytr_h, "yti": yti_h,
              "y_all": y_all, "ident": ident}

    in_maps = []
    for r in range(c.NC):
        mdict = dict(shared)
        rsl = slice(r * c.NL, (r + 1) * c.NL)
        rre_t = np.ascontiguousarray(np.asarray(R_re[rsl], f32).T)
        rim_t = np.ascontiguousarray(np.asarray(R_im[rsl], f32).T)
        parts = []
        for arr in (rre_t, rim_t, rre_t - rim_t):   # s = re - im (conj-Karatsuba)
            h, l = _split16(arr)
            for a in (h, l):
                parts.append(np.ascontiguousarray(
                    a.reshape(DT, P, c.NL).transpose(1, 0, 2).reshape(P, DT * c.NL)))
        mdict["rt_all"] = np.ascontiguousarray(np.concatenate(parts, axis=1))
        in_maps.append(mdict)
    return in_maps


_NC_CACHE = {}


def kernel(**inputs) -> np.ndarray:
    cfg = Cfg()
    if "full" not in _NC_CACHE:
        _NC_CACHE["full"] = build(cfg, 1)
    nc = _NC_CACHE["full"]
    in_maps = prep_inputs(cfg, **inputs)
    res = run_bass_kernel_spmd(nc, in_maps, list(range(cfg.NC)))
    outs = []
    for r in range(cfg.NC):
        o_re = res.results[r]["o_ret"].reshape(cfg.D, cfg.NL).T
        o_im = res.results[r]["o_imt"].reshape(cfg.D, cfg.NL).T
        outs.append(o_re + 1j * o_im)
    return np.concatenate(outs, axis=0).astype(np.complex64)
